# revision 47
# baseline (speedup 1.0000x reference)
"""Causal self-attention MLA (GQA, latent kv) kernel for 8 Trainium2 cores.

Sharding: the 8 cores map to (batch b, kv-group g) pairs: core = b*4 + g.
Each core computes, for its batch and its kv head (4 q-heads):
  qT = Wq_g^T x^T (rope)
  kT = (Wc Wk_g)^T x^T (rope),  vT = (Wc Wv_g)^T x^T   [latent proj fused on
      host: exact in real arithmetic]
  flash attention entirely in the transposed domain (fp16):
    ST[k,q] = kT^T qT  (per 128-k-block, causal blocks only)
    PT = exp(SCALE'*ST + keybias)
    causal zeroing of the diagonal 128x128 via fp16 0/1 mask multiply
    yT[d,q] += v[kb]^T PT
    PTsum   += PT; rowsum = (32*ones)^T PTsum via one matmul per head-chunk
  out_partial = yTn^T Wo_g  (row-parallel out proj)
Host sums the 4 partials per batch and scales by 1/512.

GEMM precision: the three dense projections (q, fused kv, out) run as
*compensated fp8e4 DoubleRow* matmuls: operands host-pre-scaled into
e4m3's sweet spot (x*4, W*64, y*8) and split into hi = fp8(a) and
lo = fp8(a - hi) planes.  Per two 128-k-tiles: 3 DR matmuls (hi*hi
packed over both k-tiles + one (hi,lo)x(lo,hi) cross DR per k-tile),
all in one fp32 PSUM group -> 0.75x the fp16 cycles at ~1.7e-3 end to
end error.  Scores and av stay fp16 (fp8 on the q/k/P path costs ~5%).

DMA: one serial DMA device in the cost model, so ORDER is everything.
Round 0 streams: SP carries x0 (hi quarters then lo), wq heads 1-3,
cos/sin, x1, wo -- FIFO gives natural back-pressure; ACT carries
k/v + q0 weights up-front (per-plane contiguous, elem>=512B, a tiny
first slice so the PE starts ~2us) and is free for PSUM evacs early;
Pool moves the tiny consts.  Projection passes emit all hi*hi mains
first (x-hi only), then cross DRs quarter-by-quarter as x-lo lands,
k's first so kT closes earliest for the rope.
"""
import numpy as np
import ml_dtypes

import concourse.bacc as bacc
import concourse.mybir as mybir
import concourse.tile as tile
from concourse.bass_utils import run_bass_kernel_spmd

B, L, HID = 2, 2048, 2048
NH, NKV, HD = 16, 4, 128
LAT = 512
QPG = NH // NKV            # q heads per kv group = 4
SCALE = float(HD) ** -0.5
ROPE_THETA = 10000.0
P = 128
NT = L // 512              # 4 token chunks of 512
KT = HID // P              # 16 contraction tiles
NG = KT // 2               # 8 DoubleRow k-tile pair groups
TT = L // P                # 16 token tiles of 128

SX, SW, SY = 4.0, 64.0, 8.0          # fp8 pre-scales (x, weights, y)
OUT_SCALE = SY * SW                  # host divides partials by this
F8NP = ml_dtypes.float8_e4m3

dt = mybir.dt
f32, f16, f8 = dt.float32, dt.float16, dt.float8e4
DR = mybir.MatmulPerfMode.DoubleRow

_CACHE = {}


def _build():
    nc = bacc.Bacc("TRN2", target_bir_lowering=False, debug=False)

    # hi/lo fp8 planes: weight-likes are [hi, lo], x-likes [lo, hi] so a
    # single (plane 0:2) AP pairs Wh*xl + Wl*xh for the cross-term DR
    xq_d = nc.dram_tensor("xq", [P, 2, KT, L], f8, kind="ExternalInput")
    wq_d = nc.dram_tensor("wq", [P, 2, QPG, KT, HD], f8,
                          kind="ExternalInput")
    wkv_d = nc.dram_tensor("wkv", [P, 2, 2, KT, HD], f8,
                           kind="ExternalInput")
    wo_d = nc.dram_tensor("wo", [P, 2, QPG, HID], f8, kind="ExternalInput")
    cos_d = nc.dram_tensor("cos2", [P, L], f16, kind="ExternalInput")
    sin_d = nc.dram_tensor("sin2", [P, L], f16, kind="ExternalInput")
    cpk_d = nc.dram_tensor("cpack", [P, 3 * P], f16, kind="ExternalInput")
    kb_d = nc.dram_tensor("keybias", [P, TT], f32, kind="ExternalInput")
    out_d = nc.dram_tensor("out", [L, HID], f16, kind="ExternalOutput")

    with tile.TileContext(nc) as tc:
        with tc.tile_pool(name="consts", bufs=1) as cp, \
             tc.tile_pool(name="qt", bufs=1) as qtp, \
             tc.tile_pool(name="yt", bufs=1) as ytp, \
             tc.tile_pool(name="kt", bufs=1) as ktp, \
             tc.tile_pool(name="vnat", bufs=1) as vnp, \
             tc.tile_pool(name="wgt", bufs=1) as wp, \
             tc.tile_pool(name="xc", bufs=2) as xp, \
             tc.tile_pool(name="ot", bufs=4) as otp:

            cos_t = cp.tile([P, L], f16)
            sin_t = cp.tile([P, L], f16)
            cpk_t = cp.tile([P, 3 * P], f16)
            kbias_t = cp.tile([P, TT], f32)
            m01_t = cpk_t[:, 0:P]          # causal 0/1 (k<=q)
            ones_t = cpk_t[:, P:2 * P]     # 32*ones (rowsum scale fold)
            idn_t = cpk_t[:, 2 * P:3 * P]

            qT = qtp.tile([P, QPG, L], f16)      # per-head 256*q, roped
            y_t = ytp.tile([P, 2, QPG, L], f8)   # normalized 8*y: [lo, hi]
            kT = ktp.tile([P, L], f16)           # kv-group 256*k, roped
            v_sb = vnp.tile([P, TT, HD], f16)    # 256*v natural [k,tile,d]
            wq_t = wp.tile([P, 2, QPG, KT, HD], f8)
            wkv_t = wp.tile([P, 2, 2, KT, HD], f8)
            wo_t = wp.tile([P, 2, QPG, HID], f8)

            # unified PSUM pool: projection + attention share the 8 banks
            ps_cm = tc.tile_pool(name="ps", bufs=8, space="PSUM")
            ps = ps_cm.__enter__()

            with tc.tile_pool(name="vt", bufs=2) as vtp, \
                 tc.tile_pool(name="rtmp", bufs=4) as rtp, \
                 tc.tile_pool(name="pt", bufs=8) as ptp, \
                 tc.tile_pool(name="pts", bufs=4) as psp, \
                 tc.tile_pool(name="rc", bufs=3) as rcp:

                x_tiles = {}

                def load_x(t, eng=None, planes=(1, 0), quarters=range(4)):
                    eng = eng or nc.sync
                    c0 = t * 512
                    if t not in x_tiles:
                        x_tiles[t] = xp.tile([P, 2, KT, 512], f8, tag="x",
                                             name=f"x{t}")
                    xt = x_tiles[t]
                    for g in quarters:
                        for pl in planes:
                            eng.dma_start(
                                xt[:, pl, 4 * g:4 * g + 4, :],
                                xq_d[:, pl, 4 * g:4 * g + 4, c0:c0 + 512])

                # ---- round-0 DMA streams (one serial DMA device: order
                # is everything; ACT must be free for evacs by ~6us) ----
                nc.gpsimd.dma_start(cpk_t[:], cpk_d[:])
                nc.gpsimd.dma_start(kbias_t[:], kb_d[:])
                # SP: x0 hi quarters (mains), x0 lo (crosses), rope consts,
                # then x1, then wo -- FIFO gives natural back-pressure
                xt0 = xp.tile([P, 2, KT, 512], f8, tag="x", name="x0")
                x_tiles[0] = xt0
                # tiny first slices so the PE's first main starts ~2us
                nc.sync.dma_start(xt0[:, 1, 0:2, :], xq_d[:, 1, 0:2, 0:512])
                nc.sync.dma_start(xt0[:, 1, 2:4, :], xq_d[:, 1, 2:4, 0:512])
                for g in range(1, 4):
                    nc.sync.dma_start(
                        xt0[:, 1, 4 * g:4 * g + 4, :],
                        xq_d[:, 1, 4 * g:4 * g + 4, 0:512])
                for g in range(4):
                    nc.sync.dma_start(
                        xt0[:, 0, 4 * g:4 * g + 4, :],
                        xq_d[:, 0, 4 * g:4 * g + 4, 0:512])
                for pl in range(2):
                    nc.sync.dma_start(wq_t[:, pl, 1], wq_d[:, pl, 1])
                nc.sync.dma_start(cos_t[:], cos_d[:])
                nc.sync.dma_start(sin_t[:], sin_d[:])
                for h in range(2, QPG):
                    for pl in range(2):
                        nc.sync.dma_start(wq_t[:, pl, h], wq_d[:, pl, h])
                load_x(1)
                for h in range(QPG):
                    for pl in range(2):
                        nc.sync.dma_start(wo_t[:, pl, h:h + 1, :],
                                          wo_d[:, pl, h:h + 1, :])
                # ACT: k/v hi weights (k kt0-1 first: the PE's first
                # operand), then rope consts (rope gates first attention),
                # then q0 hi and the lo planes
                nc.scalar.dma_start(wkv_t[:, 0, 0, 0:2], wkv_d[:, 0, 0, 0:2])
                nc.scalar.dma_start(wkv_t[:, 0, 0, 2:KT],
                                    wkv_d[:, 0, 0, 2:KT])
                nc.scalar.dma_start(wkv_t[:, 0, 1], wkv_d[:, 0, 1])
                nc.scalar.dma_start(wq_t[:, 0, 0], wq_d[:, 0, 0])
                nc.scalar.dma_start(wkv_t[:, 1], wkv_d[:, 1])
                nc.scalar.dma_start(wq_t[:, 1, 0], wq_d[:, 1, 0])

                def dr_mains(psum, w_hi, xt, start):
                    """hi*hi DR per k-tile pair; w_hi: g -> [128,2,HD] AP."""
                    for g in range(NG):
                        nc.tensor.matmul(
                            psum, w_hi(g), xt[:, 1, 2 * g:2 * g + 2, :],
                            start=(start and g == 0), stop=False,
                            perf_mode=DR)

                def dr_cross(psum, w_pair, xt, kts, stop_at=None):
                    """cross DR per k-tile; w_pair: kt -> [128,2,HD] AP."""
                    for kt in kts:
                        nc.tensor.matmul(
                            psum, w_pair(kt), xt[:, 0:2, kt, :],
                            start=False, stop=(kt == stop_at),
                            perf_mode=DR)

                def q_hi(h):
                    return lambda g: wq_t[:, 0, h, 2 * g:2 * g + 2, :]

                def q_pair(h):
                    return lambda kt: wq_t[:, 0:2, h, kt, :]

                def kv_hi(i):
                    return lambda g: wkv_t[:, 0, i, 2 * g:2 * g + 2, :]

                def kv_pair(i):
                    return lambda kt: wkv_t[:, 0:2, i, kt, :]

                def rope_chunk(dst, t, eng=None):
                    """In-place rope of dst[:, t*512:(t+1)*512] (fp16)."""
                    if eng is None:
                        eng = nc.vector
                    tg, nb = ("rt", 4) if eng is nc.vector else ("rtpool", 8)
                    c0, c1 = t * 512, (t + 1) * 512
                    t1c = rtp.tile([64, 512], f16, tag=tg, bufs=nb)
                    t1s = rtp.tile([64, 512], f16, tag=tg, bufs=nb)
                    t2c = rtp.tile([64, 512], f16, tag=tg, bufs=nb)
                    t2s = rtp.tile([64, 512], f16, tag=tg, bufs=nb)
                    eng.tensor_mul(t1c[:], dst[0:64, c0:c1], cos_t[0:64, c0:c1])
                    eng.tensor_mul(t1s[:], dst[0:64, c0:c1], sin_t[0:64, c0:c1])
                    eng.tensor_mul(t2c[:], dst[64:128, c0:c1],
                                   cos_t[64:128, c0:c1])
                    eng.tensor_mul(t2s[:], dst[64:128, c0:c1],
                                   sin_t[64:128, c0:c1])
                    eng.tensor_sub(dst[0:64, c0:c1], t1c[:], t2s[:])
                    eng.tensor_add(dst[64:128, c0:c1], t2c[:], t1s[:])

                def proj_pass_a(t, defer=None):
                    """k, v, q0: all hi*hi mains first (x-hi only), then
                    crosses quarter-by-quarter as x-lo lands; k's crosses
                    lead each quarter so kT closes earliest (rope dep)."""
                    xt = x_tiles[t]
                    c0, c1 = t * 512, (t + 1) * 512
                    kps = ps.tile([P, 512], f32, tag="pj", bufs=3,
                                  name=f"kps{t}")
                    vps = ps.tile([P, 512], f32, tag="pj", bufs=3,
                                  name=f"vps{t}")
                    q0ps = ps.tile([P, 512], f32, tag="pj", bufs=3,
                                   name=f"qA{t}")
                    for g in range(NG):
                        st = (g == 0)
                        nc.tensor.matmul(
                            kps[:], kv_hi(0)(g), xt[:, 1, 2 * g:2 * g + 2, :],
                            start=st, stop=False, perf_mode=DR)
                        nc.tensor.matmul(
                            vps[:], kv_hi(1)(g), xt[:, 1, 2 * g:2 * g + 2, :],
                            start=st, stop=False, perf_mode=DR)
                        nc.tensor.matmul(
                            q0ps[:], q_hi(0)(g), xt[:, 1, 2 * g:2 * g + 2, :],
                            start=st, stop=False, perf_mode=DR)
                        if g == 0 and defer is not None:
                            defer()
                    for q in range(4):
                        kts = range(4 * q, 4 * q + 4)
                        dr_cross(kps[:], kv_pair(0), xt, kts,
                                 stop_at=15 if q == 3 else None)
                        dr_cross(vps[:], kv_pair(1), xt, kts,
                                 stop_at=15 if q == 3 else None)
                        dr_cross(q0ps[:], q_pair(0), xt, kts,
                                 stop_at=15 if q == 3 else None)
                    nc.scalar.copy(kT[:, c0:c1], kps[:])
                    vt = vtp.tile([P, 512], f16, tag="vt")
                    nc.scalar.copy(vt[:], vps[:])
                    rope_chunk(kT, t)
                    nc.scalar.copy(qT[:, 0, c0:c1], q0ps[:])
                    rope_chunk(qT[:, 0, :], t)
                    return vt

                def proj_pass_b1(t):
                    xt = x_tiles[t]
                    c0, c1 = t * 512, (t + 1) * 512
                    qps = ps.tile([P, 512], f32, tag="pj", bufs=3,
                                  name=f"qB{t}_0")
                    dr_mains(qps[:], q_hi(1), xt, True)
                    dr_cross(qps[:], q_pair(1), xt, range(KT),
                             stop_at=KT - 1)
                    nc.scalar.copy(qT[:, 1, c0:c1], qps[:])
                    rope_chunk(qT[:, 1, :], t)

                def proj_pass_b23(t, defer=None):
                    xt = x_tiles[t]
                    c0, c1 = t * 512, (t + 1) * 512
                    qps = [ps.tile([P, 512], f32, tag="pj", bufs=3,
                                   name=f"qB{t}_{1 + i}") for i in range(2)]
                    for i, h in enumerate((2, 3)):
                        dr_mains(qps[i][:], q_hi(h), xt, True)
                        if i == 0 and defer is not None:
                            defer()
                    for i, h in enumerate((2, 3)):
                        dr_cross(qps[i][:], q_pair(h), xt, range(KT),
                                 stop_at=KT - 1)
                    for i, h in enumerate((2, 3)):
                        nc.scalar.copy(qT[:, h, c0:c1], qps[i][:])
                    rope_chunk(qT[:, 2, :], t,
                               eng=nc.vector if t == 0 else nc.gpsimd)
                    rope_chunk(qT[:, 3, :], t, eng=nc.gpsimd)

                def v_transposes(t, vt):
                    tp = ps.tile([P, 4, HD], f16, tag="p1", bufs=3,
                                 name=f"tp{t}")
                    for s in range(4):
                        nc.tensor.transpose(tp[:, s, :],
                                            vt[:, s * P:(s + 1) * P], idn_t)
                    nc.scalar.copy(v_sb[:, t * 4:t * 4 + 4, :], tp[:])

                def op_matmuls(o_ps, tt, oc, pairs=(0, 1), start=True,
                               stop=True):
                    """Out-proj DRs for head pairs into o_ps [128,512]."""
                    t0, t1 = tt * P, (tt + 1) * P
                    c0, c1 = oc * 512, (oc + 1) * 512
                    for i, p_ in enumerate(pairs):
                        h0 = 2 * p_
                        nc.tensor.matmul(
                            o_ps[:], y_t[:, 1, h0:h0 + 2, t0:t1],
                            wo_t[:, 0, h0:h0 + 2, c0:c1],
                            start=(start and i == 0), stop=False,
                            perf_mode=DR)
                        nc.tensor.matmul(
                            o_ps[:], y_t[:, 0:2, h0, t0:t1],
                            wo_t[:, 0:2, h0, c0:c1],
                            start=False, stop=False, perf_mode=DR)
                        nc.tensor.matmul(
                            o_ps[:], y_t[:, 0:2, h0 + 1, t0:t1],
                            wo_t[:, 0:2, h0 + 1, c0:c1],
                            start=False,
                            stop=(stop and i == len(pairs) - 1),
                            perf_mode=DR)

                def attn_chunk(qc, op_tts=(), heads=tuple(range(QPG)),
                               fin_in=None):
                    """Returns the deferred finisher for the last head.
                    op_tts: previous chunks' out-proj token-tiles as exp-free
                    PE filler, spread over this chunk's heads."""
                    q0 = qc * 512
                    nkb = 4 * qc + 4
                    units = [(tt, oc) for tt in op_tts for oc in range(4)]
                    if units:
                        n = len(units) // len(heads)
                        uph = {h: n for h in heads}
                    else:
                        uph = {h: 0 for h in heads}
                    ustate = {"i": 0, "ot": None, "tt": None}

                    def make_fin(h, y_ps, ptsum, ptsum2=None):
                        def fin():
                            rs_ps = ps.tile([P, 512], f32, tag="p1", bufs=3,
                                            name=f"rsps{qc}_{h}")
                            nc.tensor.matmul(rs_ps[:], ones_t, ptsum[:],
                                             start=True,
                                             stop=(ptsum2 is None))
                            if ptsum2 is not None:
                                nc.tensor.matmul(rs_ps[:], ones_t,
                                                 ptsum2[:],
                                                 start=False, stop=True)
                            rec = rcp.tile([P, 512], f32, tag="rc")
                            nc.vector.reciprocal(rec[:], rs_ps[:])
                            y16 = rcp.tile([P, 512], f16, tag="y16", bufs=2)
                            nc.vector.tensor_mul(y16[:], y_ps[:], rec[:])
                            nc.vector.tensor_copy(
                                y_t[:, 1, h, q0:q0 + 512], y16[:])
                            nc.vector.tensor_sub(
                                y_t[:, 0, h, q0:q0 + 512], y16[:],
                                y_t[:, 1, h, q0:q0 + 512])
                        return fin

                    def emit_unit():
                        i = ustate["i"]
                        if i >= len(units):
                            return
                        ustate["i"] = i + 1
                        tt, oc = units[i]
                        if oc == 0:
                            ustate["ot"] = otp.tile([P, HID], f16, tag="ot",
                                                    name=f"ot{tt}")
                        ot = ustate["ot"]
                        o_ps = ps.tile([P, 512], f32, tag="pj", bufs=3,
                                       name=f"ops{tt}_{oc}")
                        op_matmuls(o_ps, tt, oc)
                        act_heavy = (qc == NT - 1
                                     and ustate.get("h") == QPG - 1)
                        if oc == 0 or (act_heavy and oc != 2):
                            nc.scalar.copy(
                                ot[:, oc * 512:(oc + 1) * 512], o_ps[:])
                        else:
                            nc.vector.tensor_copy(
                                ot[:, oc * 512:(oc + 1) * 512], o_ps[:])
                        if oc == 1:
                            nc.sync.dma_start(
                                out_d[tt * P:(tt + 1) * P, 0:1024],
                                ot[:, 0:1024])
                        if oc == 3:
                            nc.sync.dma_start(
                                out_d[tt * P:(tt + 1) * P, 1024:2048],
                                ot[:, 1024:2048])

                    fin_prev = fin_in
                    for h in heads:
                        ustate["h"] = h
                        nph = uph[h]
                        span = (nkb - 2) if h == QPG - 1 else nkb
                        span = max(span, nph)
                        op_points = {(i + 1) * span // nph - 1: i
                                     for i in range(nph)} if nph else {}
                        y_ps = ps.tile([P, 512], f32, tag="py", bufs=2,
                                       name=f"yps{qc}_{h}")
                        ptsum = psp.tile([P, 512], f16, tag="pts")
                        # last chunk's last head: even-kb PTsum adds ride a
                        # Pool side-accumulator so the DVE clears its y-split
                        # quickly (the final out-proj waits on it)
                        pool_side = (qc == NT - 1 and h == QPG - 1)
                        ptsum2 = None
                        p2_first = None
                        pend = None
                        for kb in range(nkb):
                            c0 = max(0, kb * P - q0)
                            w = 512 - c0
                            st_ps = ps.tile([P, w], f32, tag="p1", bufs=3,
                                            name=f"stps{qc}_{h}_{kb}")
                            nc.tensor.matmul(
                                st_ps[:], kT[:, kb * P:(kb + 1) * P],
                                qT[:, h, q0 + c0:q0 + 512],
                                start=True, stop=True)
                            if kb == 0:
                                pt = ptsum[:, 0:512]
                            else:
                                ptt = ptp.tile([P, w], f16, tag="pt",
                                               name=f"pt{qc}_{h}_{kb}")
                                pt = ptt[:]
                            nc.scalar.activation(
                                pt, st_ps[:],
                                mybir.ActivationFunctionType.Exp,
                                bias=kbias_t[:, kb:kb + 1],
                                scale=SCALE / 65536.0)
                            if kb >= 4 * qc:  # diagonal: zero upper triangle
                                meng = (nc.gpsimd
                                        if qc == NT - 1 and h == 2
                                        else nc.vector)
                                meng.tensor_mul(pt[:, 0:P], pt[:, 0:P],
                                                m01_t)
                            if pend is not None:
                                pkb, pc0, ppt = pend
                                nc.tensor.matmul(
                                    y_ps[:, pc0:512], v_sb[:, pkb, :], ppt,
                                    start=(pkb == 0), stop=False)
                            if kb in op_points:
                                emit_unit()
                            if kb > 0:
                                if pool_side and kb % 2 == 0 and w == 512:
                                    if p2_first is None:
                                        p2_first = pt
                                    elif ptsum2 is None:
                                        ptsum2 = psp.tile(
                                            [P, 512], f16, tag="pts2",
                                            bufs=2, name=f"p2_{qc}_{h}")
                                        nc.gpsimd.tensor_add(
                                            ptsum2[:], p2_first, pt)
                                    else:
                                        nc.gpsimd.tensor_add(
                                            ptsum2[:], ptsum2[:], pt)
                                else:
                                    nc.vector.tensor_add(
                                        ptsum[:, c0:512], ptsum[:, c0:512],
                                        pt)
                            pend = (kb, c0, pt)
                            if kb == 1 and fin_prev is not None:
                                fin_prev()
                                fin_prev = None
                        if pool_side and p2_first is not None \
                                and ptsum2 is None:
                            ptsum2 = psp.tile([P, 512], f16, tag="pts2",
                                              bufs=2, name=f"p2_{qc}_{h}")
                            nc.gpsimd.tensor_copy(ptsum2[:], p2_first)
                        pkb, pc0, ppt = pend
                        nc.tensor.matmul(
                            y_ps[:, pc0:512], v_sb[:, pkb, :], ppt,
                            start=(pkb == 0), stop=True)
                        fin_prev = make_fin(h, y_ps, ptsum, ptsum2)
                    return fin_prev

                def outproj_tt(tt, defer=None, last=False):
                        ot = otp.tile([P, HID], f16, tag="ot")
                        if defer is not None:
                            # first head-pair for three oc tiles first; the
                            # deferred last-head softmax finish runs under
                            # their PE cover, then pair 1 joins
                            opss = {}

                            def op_p0(oc):
                                o_ps = ps.tile([P, 512], f32, tag="pj",
                                               bufs=3, name=f"ops{tt}_{oc}")
                                opss[oc] = o_ps
                                op_matmuls(o_ps, tt, oc, pairs=(0,),
                                           start=True, stop=False)

                            def op_p1(oc):
                                op_matmuls(opss[oc], tt, oc, pairs=(1,),
                                           start=False, stop=True)
                                if oc % 2 == 0:
                                    nc.scalar.copy(
                                        ot[:, oc * 512:(oc + 1) * 512],
                                        opss[oc][:])
                                else:
                                    nc.vector.tensor_copy(
                                        ot[:, oc * 512:(oc + 1) * 512],
                                        opss[oc][:])

                            op_p0(0)
                            op_p0(1)
                            defer()
                            op_p0(2)
                            op_p1(0)
                            op_p1(1)
                            op_p0(3)
                            op_p1(2)
                            op_p1(3)
                        else:
                            for oc in range(4):
                                o_ps = ps.tile([P, 512], f32, tag="pj",
                                               bufs=3, name=f"ops{tt}_{oc}")
                                op_matmuls(o_ps, tt, oc)
                                if oc > 0:
                                    nc.vector.tensor_copy(
                                        ot[:, oc * 512:(oc + 1) * 512],
                                        o_ps[:])
                                else:
                                    nc.scalar.copy(
                                        ot[:, oc * 512:(oc + 1) * 512],
                                        o_ps[:])
                        if last and tt % 4 == 3:
                            for oc in range(4):
                                q = nc.sync if oc % 2 == 0 else nc.scalar
                                q.dma_start(
                                    out_d[tt * P:(tt + 1) * P,
                                          oc * 512:(oc + 1) * 512],
                                    ot[:, oc * 512:(oc + 1) * 512])
                        else:
                            nc.sync.dma_start(
                                out_d[tt * P:(tt + 1) * P, 0:1024],
                                ot[:, 0:1024])
                            nc.sync.dma_start(
                                out_d[tt * P:(tt + 1) * P, 1024:2048],
                                ot[:, 1024:2048])

                # round structure: projA/B(t) -> transposes -> prefetch
                # x(t+1) -> outproj(t-1) -> attn(t)
                OP_SCHED = [(), (0, 1, 2), (3, 4, 5), (6, 7, 8, 9, 10, 11)]
                fin = None
                for t in range(NT):
                    if 0 < t < NT - 1:
                        load_x(t + 1)
                    vt = proj_pass_a(t, defer=fin)
                    proj_pass_b1(t)
                    proj_pass_b23(t)
                    v_transposes(t, vt)
                    fin = attn_chunk(t, OP_SCHED[t])
                for tt in range((NT - 1) * 4, NT * 4):
                    outproj_tt(tt, defer=fin, last=True)
                    fin = None

            ps_cm.__exit__(None, None, None)

    nc.compile()
    return nc


def _host_consts(attention_mask):
    half = HD // 2
    inv_freq = (1.0 / (ROPE_THETA ** (np.arange(half, dtype=np.float32) / half))
                ).astype(np.float32)
    pos = np.arange(L, dtype=np.float32)
    freqs = pos[None, :] * inv_freq[:, None]          # [64, L]
    cos = np.cos(freqs).astype(np.float16)
    sin = np.sin(freqs).astype(np.float16)
    cos2 = np.ascontiguousarray(np.concatenate([cos, cos], axis=0))
    sin2 = np.ascontiguousarray(np.concatenate([sin, sin], axis=0))
    k_idx = np.arange(P)[:, None]
    q_idx = np.arange(P)[None, :]
    m01 = (k_idx <= q_idx).astype(np.float16)
    onesm = np.full((P, P), 32.0, np.float16)
    ident = np.eye(P, dtype=np.float16)
    cpack = np.ascontiguousarray(
        np.concatenate([m01, onesm, ident], axis=1))
    kbias = []
    for b in range(B):
        m = attention_mask[b].astype(np.float32)      # [L]
        bias = np.where(m > 0, 0.0, -1e4).astype(np.float32)
        kbias.append(np.ascontiguousarray(bias.reshape(TT, P).T))
    return cos2, sin2, cpack, kbias


def _hl(a):
    hi = a.astype(F8NP)
    lo = (a - hi.astype(np.float32)).astype(F8NP)
    return hi, lo


def _lay_x(a):
    """[HID, L] scaled -> [P, 2, KT, L] planes (lo, hi)."""
    hi, lo = _hl(a)

    def lay(arr):
        return arr.reshape(KT, P, L).transpose(1, 0, 2)

    return np.ascontiguousarray(np.stack([lay(lo), lay(hi)], axis=1))


def _lay_wq(a):
    """[HID, QPG*HD] scaled -> [P, 2, QPG, KT, HD] planes (hi, lo)."""
    hi, lo = _hl(a)

    def lay(arr):
        return arr.reshape(KT, P, QPG, HD).transpose(1, 2, 0, 3)

    return np.ascontiguousarray(np.stack([lay(hi), lay(lo)], axis=1))


def _lay_wkv(a):
    """[HID, 2*HD] scaled -> [P, 2, 2, KT, HD] planes (hi, lo)."""
    hi, lo = _hl(a)

    def lay(arr):
        return arr.reshape(KT, P, 2, HD).transpose(1, 2, 0, 3)

    return np.ascontiguousarray(np.stack([lay(hi), lay(lo)], axis=1))


def _lay_wo(a):
    """[QPG*HD, HID] scaled -> [P, 2, QPG, HID] planes (hi, lo)."""
    hi, lo = _hl(a)

    def lay(arr):
        return arr.reshape(QPG, P, HID).transpose(1, 0, 2)

    return np.ascontiguousarray(np.stack([lay(hi), lay(lo)], axis=1))


def kernel(x, Wq, Wc, Wk, Wv, Wo, attention_mask):
    x = np.asarray(x, dtype=np.float32)
    Wq = np.asarray(Wq, dtype=np.float32)
    Wc = np.asarray(Wc, dtype=np.float32)
    Wk = np.asarray(Wk, dtype=np.float32)
    Wv = np.asarray(Wv, dtype=np.float32)
    Wo = np.asarray(Wo, dtype=np.float32)
    attention_mask = np.asarray(attention_mask)

    if "nc" not in _CACHE:
        _CACHE["nc"] = _build()
    nc = _CACHE["nc"]

    cos2, sin2, cpack, kbias = _host_consts(attention_mask)
    # fuse the latent projection on host (exact up to fp rounding)
    Wck = (Wc.astype(np.float64) @ Wk.astype(np.float64)).astype(np.float32)
    Wcv = (Wc.astype(np.float64) @ Wv.astype(np.float64)).astype(np.float32)

    xq = [_lay_x(SX * x[b].T) for b in range(B)]

    in_maps = []
    for core in range(8):
        b, g = core // QPG, core % QPG
        wkv = np.concatenate(
            [Wck[:, g * HD:(g + 1) * HD], Wcv[:, g * HD:(g + 1) * HD]],
            axis=1)
        in_maps.append({
            "xq": xq[b],
            "wq": _lay_wq(SW * Wq[:, g * QPG * HD:(g + 1) * QPG * HD]),
            "wkv": _lay_wkv(SW * wkv),
            "wo": _lay_wo(SW * Wo[g * QPG * HD:(g + 1) * QPG * HD, :]),
            "cos2": cos2, "sin2": sin2, "cpack": cpack, "keybias": kbias[b],
        })

    res = run_bass_kernel_spmd(nc, in_maps, core_ids=list(range(8)))
    out = np.zeros((B, L, HID), dtype=np.float32)
    for core in range(8):
        out[core // QPG] += res.results[core]["out"].astype(np.float32)
    out *= 1.0 / OUT_SCALE
    return out


# revision 48
# speedup vs baseline: 1.0065x; 1.0065x over previous
"""Causal self-attention MLA (GQA, latent kv) kernel for 8 Trainium2 cores.

Sharding: the 8 cores map to (batch b, kv-group g) pairs: core = b*4 + g.
Each core computes, for its batch and its kv head (4 q-heads):
  qT = Wq_g^T x^T (rope)
  kT = (Wc Wk_g)^T x^T (rope),  vT = (Wc Wv_g)^T x^T   [latent proj fused on
      host: exact in real arithmetic]
  flash attention entirely in the transposed domain (fp16):
    ST[k,q] = kT^T qT  (per 128-k-block, causal blocks only)
    PT = exp(SCALE'*ST + keybias)
    causal zeroing of the diagonal 128x128 via fp16 0/1 mask multiply
    yT[d,q] += v[kb]^T PT
    PTsum   += PT; rowsum = (32*ones)^T PTsum via one matmul per head-chunk
  out_partial = yTn^T Wo_g  (row-parallel out proj)
Host sums the 4 partials per batch and scales by 1/512.

GEMM precision: the three dense projections (q, fused kv, out) run as
*compensated fp8e4 DoubleRow* matmuls: operands host-pre-scaled into
e4m3's sweet spot (x*4, W*64, y*8) and split into hi = fp8(a) and
lo = fp8(a - hi) planes.  Per two 128-k-tiles: 3 DR matmuls (hi*hi
packed over both k-tiles + one (hi,lo)x(lo,hi) cross DR per k-tile),
all in one fp32 PSUM group -> 0.75x the fp16 cycles at ~1.7e-3 end to
end error.  Scores and av stay fp16 (fp8 on the q/k/P path costs ~5%).

DMA: one serial DMA device in the cost model, so ORDER is everything.
Round 0 streams: SP carries x0 (hi quarters then lo), wq heads 1-3,
cos/sin, x1, wo -- FIFO gives natural back-pressure; ACT carries
k/v + q0 weights up-front (per-plane contiguous, elem>=512B, a tiny
first slice so the PE starts ~2us) and is free for PSUM evacs early;
Pool moves the tiny consts.  Projection passes emit all hi*hi mains
first (x-hi only), then cross DRs quarter-by-quarter as x-lo lands,
k's first so kT closes earliest for the rope.
"""
import numpy as np
import ml_dtypes

import concourse.bacc as bacc
import concourse.mybir as mybir
import concourse.tile as tile
from concourse.bass_utils import run_bass_kernel_spmd

B, L, HID = 2, 2048, 2048
NH, NKV, HD = 16, 4, 128
LAT = 512
QPG = NH // NKV            # q heads per kv group = 4
SCALE = float(HD) ** -0.5
ROPE_THETA = 10000.0
P = 128
NT = L // 512              # 4 token chunks of 512
KT = HID // P              # 16 contraction tiles
NG = KT // 2               # 8 DoubleRow k-tile pair groups
TT = L // P                # 16 token tiles of 128

SX, SW, SY = 4.0, 64.0, 8.0          # fp8 pre-scales (x, weights, y)
OUT_SCALE = SY * SW                  # host divides partials by this
F8NP = ml_dtypes.float8_e4m3

dt = mybir.dt
f32, f16, f8 = dt.float32, dt.float16, dt.float8e4
DR = mybir.MatmulPerfMode.DoubleRow

_CACHE = {}


def _build():
    nc = bacc.Bacc("TRN2", target_bir_lowering=False, debug=False)

    # hi/lo fp8 planes: weight-likes are [hi, lo], x-likes [lo, hi] so a
    # single (plane 0:2) AP pairs Wh*xl + Wl*xh for the cross-term DR
    xq_d = nc.dram_tensor("xq", [P, 2, KT, L], f8, kind="ExternalInput")
    wq_d = nc.dram_tensor("wq", [P, 2, QPG, KT, HD], f8,
                          kind="ExternalInput")
    wkv_d = nc.dram_tensor("wkv", [P, 2, 2, KT, HD], f8,
                           kind="ExternalInput")
    wo_d = nc.dram_tensor("wo", [P, 2, QPG, HID], f8, kind="ExternalInput")
    cos_d = nc.dram_tensor("cos2", [P, L], f16, kind="ExternalInput")
    sin_d = nc.dram_tensor("sin2", [P, L], f16, kind="ExternalInput")
    cpk_d = nc.dram_tensor("cpack", [P, 3 * P], f16, kind="ExternalInput")
    kb_d = nc.dram_tensor("keybias", [P, TT], f32, kind="ExternalInput")
    out_d = nc.dram_tensor("out", [L, HID], f16, kind="ExternalOutput")

    with tile.TileContext(nc) as tc:
        with tc.tile_pool(name="consts", bufs=1) as cp, \
             tc.tile_pool(name="qt", bufs=1) as qtp, \
             tc.tile_pool(name="yt", bufs=1) as ytp, \
             tc.tile_pool(name="kt", bufs=1) as ktp, \
             tc.tile_pool(name="vnat", bufs=1) as vnp, \
             tc.tile_pool(name="wgt", bufs=1) as wp, \
             tc.tile_pool(name="xc", bufs=2) as xp, \
             tc.tile_pool(name="ot", bufs=4) as otp:

            cos_t = cp.tile([P, L], f16)
            sin_t = cp.tile([P, L], f16)
            cpk_t = cp.tile([P, 3 * P], f16)
            kbias_t = cp.tile([P, TT], f32)
            m01_t = cpk_t[:, 0:P]          # causal 0/1 (k<=q)
            ones_t = cpk_t[:, P:2 * P]     # 32*ones (rowsum scale fold)
            idn_t = cpk_t[:, 2 * P:3 * P]

            qT = qtp.tile([P, QPG, L], f16)      # per-head 256*q, roped
            y_t = ytp.tile([P, 2, QPG, L], f8)   # normalized 8*y: [lo, hi]
            kT = ktp.tile([P, L], f16)           # kv-group 256*k, roped
            v_sb = vnp.tile([P, TT, HD], f16)    # 256*v natural [k,tile,d]
            wq_t = wp.tile([P, 2, QPG, KT, HD], f8)
            wkv_t = wp.tile([P, 2, 2, KT, HD], f8)
            wo_t = wp.tile([P, 2, QPG, HID], f8)

            # unified PSUM pool: projection + attention share the 8 banks
            ps_cm = tc.tile_pool(name="ps", bufs=8, space="PSUM")
            ps = ps_cm.__enter__()

            with tc.tile_pool(name="vt", bufs=2) as vtp, \
                 tc.tile_pool(name="rtmp", bufs=4) as rtp, \
                 tc.tile_pool(name="pt", bufs=8) as ptp, \
                 tc.tile_pool(name="pts", bufs=4) as psp, \
                 tc.tile_pool(name="rc", bufs=3) as rcp:

                x_tiles = {}

                def load_x(t, eng=None, planes=(1, 0), quarters=range(4)):
                    eng = eng or nc.sync
                    c0 = t * 512
                    if t not in x_tiles:
                        x_tiles[t] = xp.tile([P, 2, KT, 512], f8, tag="x",
                                             name=f"x{t}")
                    xt = x_tiles[t]
                    for g in quarters:
                        for pl in planes:
                            eng.dma_start(
                                xt[:, pl, 4 * g:4 * g + 4, :],
                                xq_d[:, pl, 4 * g:4 * g + 4, c0:c0 + 512])

                # ---- round-0 DMA streams (one serial DMA device: order
                # is everything; ACT must be free for evacs by ~6us) ----
                nc.gpsimd.dma_start(cpk_t[:], cpk_d[:])
                nc.gpsimd.dma_start(kbias_t[:], kb_d[:])
                # SP: x0 hi quarters (mains), x0 lo (crosses), rope consts,
                # then x1, then wo -- FIFO gives natural back-pressure
                xt0 = xp.tile([P, 2, KT, 512], f8, tag="x", name="x0")
                x_tiles[0] = xt0
                # tiny first slices so the PE's first main starts ~2us
                nc.sync.dma_start(xt0[:, 1, 0:2, :], xq_d[:, 1, 0:2, 0:512])
                nc.sync.dma_start(xt0[:, 1, 2:4, :], xq_d[:, 1, 2:4, 0:512])
                for g in range(1, 4):
                    nc.sync.dma_start(
                        xt0[:, 1, 4 * g:4 * g + 4, :],
                        xq_d[:, 1, 4 * g:4 * g + 4, 0:512])
                for g in range(4):
                    nc.sync.dma_start(
                        xt0[:, 0, 4 * g:4 * g + 4, :],
                        xq_d[:, 0, 4 * g:4 * g + 4, 0:512])
                for pl in range(2):
                    nc.sync.dma_start(wq_t[:, pl, 1], wq_d[:, pl, 1])
                nc.sync.dma_start(cos_t[:], cos_d[:])
                nc.sync.dma_start(sin_t[:], sin_d[:])
                for h in range(2, QPG):
                    for pl in range(2):
                        nc.sync.dma_start(wq_t[:, pl, h], wq_d[:, pl, h])
                load_x(1)
                for h in range(QPG):
                    for pl in range(2):
                        nc.sync.dma_start(wo_t[:, pl, h:h + 1, :],
                                          wo_d[:, pl, h:h + 1, :])
                # ACT: k/v hi weights (k kt0-1 first: the PE's first
                # operand), then rope consts (rope gates first attention),
                # then q0 hi and the lo planes
                nc.scalar.dma_start(wkv_t[:, 0, 0, 0:2], wkv_d[:, 0, 0, 0:2])
                nc.scalar.dma_start(wkv_t[:, 0, 0, 2:KT],
                                    wkv_d[:, 0, 0, 2:KT])
                nc.scalar.dma_start(wkv_t[:, 0, 1], wkv_d[:, 0, 1])
                nc.scalar.dma_start(wq_t[:, 0, 0], wq_d[:, 0, 0])
                nc.scalar.dma_start(wkv_t[:, 1], wkv_d[:, 1])
                nc.scalar.dma_start(wq_t[:, 1, 0], wq_d[:, 1, 0])

                def dr_mains(psum, w_hi, xt, start):
                    """hi*hi DR per k-tile pair; w_hi: g -> [128,2,HD] AP."""
                    for g in range(NG):
                        nc.tensor.matmul(
                            psum, w_hi(g), xt[:, 1, 2 * g:2 * g + 2, :],
                            start=(start and g == 0), stop=False,
                            perf_mode=DR)

                def dr_cross(psum, w_pair, xt, kts, stop_at=None):
                    """cross DR per k-tile; w_pair: kt -> [128,2,HD] AP."""
                    for kt in kts:
                        nc.tensor.matmul(
                            psum, w_pair(kt), xt[:, 0:2, kt, :],
                            start=False, stop=(kt == stop_at),
                            perf_mode=DR)

                def q_hi(h):
                    return lambda g: wq_t[:, 0, h, 2 * g:2 * g + 2, :]

                def q_pair(h):
                    return lambda kt: wq_t[:, 0:2, h, kt, :]

                def kv_hi(i):
                    return lambda g: wkv_t[:, 0, i, 2 * g:2 * g + 2, :]

                def kv_pair(i):
                    return lambda kt: wkv_t[:, 0:2, i, kt, :]

                def rope_chunk(dst, t, eng=None):
                    """In-place rope of dst[:, t*512:(t+1)*512] (fp16)."""
                    if eng is None:
                        eng = nc.vector
                    tg, nb = ("rt", 4) if eng is nc.vector else ("rtpool", 8)
                    c0, c1 = t * 512, (t + 1) * 512
                    t1c = rtp.tile([64, 512], f16, tag=tg, bufs=nb)
                    t1s = rtp.tile([64, 512], f16, tag=tg, bufs=nb)
                    t2c = rtp.tile([64, 512], f16, tag=tg, bufs=nb)
                    t2s = rtp.tile([64, 512], f16, tag=tg, bufs=nb)
                    eng.tensor_mul(t1c[:], dst[0:64, c0:c1], cos_t[0:64, c0:c1])
                    eng.tensor_mul(t1s[:], dst[0:64, c0:c1], sin_t[0:64, c0:c1])
                    eng.tensor_mul(t2c[:], dst[64:128, c0:c1],
                                   cos_t[64:128, c0:c1])
                    eng.tensor_mul(t2s[:], dst[64:128, c0:c1],
                                   sin_t[64:128, c0:c1])
                    eng.tensor_sub(dst[0:64, c0:c1], t1c[:], t2s[:])
                    eng.tensor_add(dst[64:128, c0:c1], t2c[:], t1s[:])

                def proj_pass_a(t, defer=None):
                    """k, v, q0: all hi*hi mains first (x-hi only), then
                    crosses quarter-by-quarter as x-lo lands; k's crosses
                    lead each quarter so kT closes earliest (rope dep)."""
                    xt = x_tiles[t]
                    c0, c1 = t * 512, (t + 1) * 512
                    kps = ps.tile([P, 512], f32, tag="pj", bufs=3,
                                  name=f"kps{t}")
                    vps = ps.tile([P, 512], f32, tag="pj", bufs=3,
                                  name=f"vps{t}")
                    q0ps = ps.tile([P, 512], f32, tag="pj", bufs=3,
                                   name=f"qA{t}")
                    for g in range(NG):
                        st = (g == 0)
                        nc.tensor.matmul(
                            kps[:], kv_hi(0)(g), xt[:, 1, 2 * g:2 * g + 2, :],
                            start=st, stop=False, perf_mode=DR)
                        nc.tensor.matmul(
                            vps[:], kv_hi(1)(g), xt[:, 1, 2 * g:2 * g + 2, :],
                            start=st, stop=False, perf_mode=DR)
                        nc.tensor.matmul(
                            q0ps[:], q_hi(0)(g), xt[:, 1, 2 * g:2 * g + 2, :],
                            start=st, stop=False, perf_mode=DR)
                        if g == 0 and defer is not None:
                            defer()
                    for q in range(4):
                        kts = range(4 * q, 4 * q + 4)
                        dr_cross(kps[:], kv_pair(0), xt, kts,
                                 stop_at=15 if q == 3 else None)
                        dr_cross(vps[:], kv_pair(1), xt, kts,
                                 stop_at=15 if q == 3 else None)
                        dr_cross(q0ps[:], q_pair(0), xt, kts,
                                 stop_at=15 if q == 3 else None)
                    nc.scalar.copy(kT[:, c0:c1], kps[:])
                    vt = vtp.tile([P, 512], f16, tag="vt")
                    nc.scalar.copy(vt[:], vps[:])
                    rope_chunk(kT, t)
                    nc.scalar.copy(qT[:, 0, c0:c1], q0ps[:])
                    rope_chunk(qT[:, 0, :], t)
                    return vt

                def proj_pass_b1(t):
                    xt = x_tiles[t]
                    c0, c1 = t * 512, (t + 1) * 512
                    qps = ps.tile([P, 512], f32, tag="pj", bufs=3,
                                  name=f"qB{t}_0")
                    dr_mains(qps[:], q_hi(1), xt, True)
                    dr_cross(qps[:], q_pair(1), xt, range(KT),
                             stop_at=KT - 1)
                    nc.scalar.copy(qT[:, 1, c0:c1], qps[:])
                    rope_chunk(qT[:, 1, :], t)

                def proj_pass_b23(t, defer=None):
                    xt = x_tiles[t]
                    c0, c1 = t * 512, (t + 1) * 512
                    qps = [ps.tile([P, 512], f32, tag="pj", bufs=3,
                                   name=f"qB{t}_{1 + i}") for i in range(2)]
                    for i, h in enumerate((2, 3)):
                        dr_mains(qps[i][:], q_hi(h), xt, True)
                        if i == 0 and defer is not None:
                            defer()
                    for i, h in enumerate((2, 3)):
                        dr_cross(qps[i][:], q_pair(h), xt, range(KT),
                                 stop_at=KT - 1)
                    for i, h in enumerate((2, 3)):
                        nc.scalar.copy(qT[:, h, c0:c1], qps[i][:])
                    rope_chunk(qT[:, 2, :], t,
                               eng=nc.vector if t == 0 else nc.gpsimd)
                    rope_chunk(qT[:, 3, :], t, eng=nc.gpsimd)

                def v_transposes(t, vt):
                    tp = ps.tile([P, 4, HD], f16, tag="p1", bufs=3,
                                 name=f"tp{t}")
                    for s in range(4):
                        nc.tensor.transpose(tp[:, s, :],
                                            vt[:, s * P:(s + 1) * P], idn_t)
                    nc.scalar.copy(v_sb[:, t * 4:t * 4 + 4, :], tp[:])

                def op_matmuls(o_ps, tt, oc, pairs=(0, 1), start=True,
                               stop=True):
                    """Out-proj DRs for head pairs into o_ps [128,512]."""
                    t0, t1 = tt * P, (tt + 1) * P
                    c0, c1 = oc * 512, (oc + 1) * 512
                    for i, p_ in enumerate(pairs):
                        h0 = 2 * p_
                        nc.tensor.matmul(
                            o_ps[:], y_t[:, 1, h0:h0 + 2, t0:t1],
                            wo_t[:, 0, h0:h0 + 2, c0:c1],
                            start=(start and i == 0), stop=False,
                            perf_mode=DR)
                        nc.tensor.matmul(
                            o_ps[:], y_t[:, 0:2, h0, t0:t1],
                            wo_t[:, 0:2, h0, c0:c1],
                            start=False, stop=False, perf_mode=DR)
                        nc.tensor.matmul(
                            o_ps[:], y_t[:, 0:2, h0 + 1, t0:t1],
                            wo_t[:, 0:2, h0 + 1, c0:c1],
                            start=False,
                            stop=(stop and i == len(pairs) - 1),
                            perf_mode=DR)

                def attn_chunk(qc, op_tts=(), heads=tuple(range(QPG)),
                               fin_in=None):
                    """Returns the deferred finisher for the last head.
                    op_tts: previous chunks' out-proj token-tiles as exp-free
                    PE filler, spread over this chunk's heads."""
                    q0 = qc * 512
                    nkb = 4 * qc + 4
                    units = [(tt, oc) for tt in op_tts for oc in range(4)]
                    if units:
                        n = len(units) // len(heads)
                        uph = {h: n for h in heads}
                    else:
                        uph = {h: 0 for h in heads}
                    ustate = {"i": 0, "ot": None, "tt": None}

                    def make_fin(h, y_ps, ptsum, ptsum2=None):
                        def fin():
                            rs_ps = ps.tile([P, 512], f32, tag="p1", bufs=3,
                                            name=f"rsps{qc}_{h}")
                            nc.tensor.matmul(rs_ps[:], ones_t, ptsum[:],
                                             start=True,
                                             stop=(ptsum2 is None))
                            if ptsum2 is not None:
                                nc.tensor.matmul(rs_ps[:], ones_t,
                                                 ptsum2[:],
                                                 start=False, stop=True)
                            rec = rcp.tile([P, 512], f32, tag="rc")
                            nc.vector.reciprocal(rec[:], rs_ps[:])
                            y16 = rcp.tile([P, 512], f16, tag="y16", bufs=2)
                            nc.vector.tensor_mul(y16[:], y_ps[:], rec[:])
                            nc.vector.tensor_copy(
                                y_t[:, 1, h, q0:q0 + 512], y16[:])
                            nc.vector.tensor_sub(
                                y_t[:, 0, h, q0:q0 + 512], y16[:],
                                y_t[:, 1, h, q0:q0 + 512])
                        return fin

                    def emit_unit():
                        i = ustate["i"]
                        if i >= len(units):
                            return
                        ustate["i"] = i + 1
                        tt, oc = units[i]
                        if oc == 0:
                            ustate["ot"] = otp.tile([P, HID], f16, tag="ot",
                                                    name=f"ot{tt}")
                        ot = ustate["ot"]
                        o_ps = ps.tile([P, 512], f32, tag="pj", bufs=3,
                                       name=f"ops{tt}_{oc}")
                        op_matmuls(o_ps, tt, oc)
                        act_heavy = (qc == NT - 1
                                     and ustate.get("h") == QPG - 1)
                        if oc == 0 or (act_heavy and oc != 2):
                            nc.scalar.copy(
                                ot[:, oc * 512:(oc + 1) * 512], o_ps[:])
                        else:
                            nc.vector.tensor_copy(
                                ot[:, oc * 512:(oc + 1) * 512], o_ps[:])
                        if oc == 1:
                            nc.sync.dma_start(
                                out_d[tt * P:(tt + 1) * P, 0:1024],
                                ot[:, 0:1024])
                        if oc == 3:
                            nc.sync.dma_start(
                                out_d[tt * P:(tt + 1) * P, 1024:2048],
                                ot[:, 1024:2048])

                    fin_prev = fin_in
                    for h in heads:
                        ustate["h"] = h
                        nph = uph[h]
                        span = (nkb - 2) if h == QPG - 1 else nkb
                        span = max(span, nph)
                        op_points = {(i + 1) * span // nph - 1: i
                                     for i in range(nph)} if nph else {}
                        y_ps = ps.tile([P, 512], f32, tag="py", bufs=2,
                                       name=f"yps{qc}_{h}")
                        ptsum = psp.tile([P, 512], f16, tag="pts")
                        # last chunk's last head: even-kb PTsum adds ride a
                        # Pool side-accumulator so the DVE clears its y-split
                        # quickly (the final out-proj waits on it)
                        pool_side = (qc == NT - 1 and h == QPG - 1)
                        ptsum2 = None
                        p2_first = None
                        pend = None
                        for kb in range(nkb):
                            c0 = max(0, kb * P - q0)
                            w = 512 - c0
                            st_ps = ps.tile([P, w], f32, tag="p1", bufs=3,
                                            name=f"stps{qc}_{h}_{kb}")
                            nc.tensor.matmul(
                                st_ps[:], kT[:, kb * P:(kb + 1) * P],
                                qT[:, h, q0 + c0:q0 + 512],
                                start=True, stop=True)
                            if kb == 0:
                                pt = ptsum[:, 0:512]
                            else:
                                ptt = ptp.tile([P, w], f16, tag="pt",
                                               name=f"pt{qc}_{h}_{kb}")
                                pt = ptt[:]
                            nc.scalar.activation(
                                pt, st_ps[:],
                                mybir.ActivationFunctionType.Exp,
                                bias=kbias_t[:, kb:kb + 1],
                                scale=SCALE / 65536.0)
                            if kb >= 4 * qc:  # diagonal: zero upper triangle
                                meng = (nc.gpsimd
                                        if qc == NT - 1 and h == 2
                                        else nc.vector)
                                meng.tensor_mul(pt[:, 0:P], pt[:, 0:P],
                                                m01_t)
                            if pend is not None:
                                pkb, pc0, ppt = pend
                                nc.tensor.matmul(
                                    y_ps[:, pc0:512], v_sb[:, pkb, :], ppt,
                                    start=(pkb == 0), stop=False)
                            if kb in op_points:
                                emit_unit()
                            if kb > 0:
                                if pool_side and kb % 2 == 0 and w == 512:
                                    if p2_first is None:
                                        p2_first = pt
                                    elif ptsum2 is None:
                                        ptsum2 = psp.tile(
                                            [P, 512], f16, tag="pts2",
                                            bufs=2, name=f"p2_{qc}_{h}")
                                        nc.gpsimd.tensor_add(
                                            ptsum2[:], p2_first, pt)
                                    else:
                                        nc.gpsimd.tensor_add(
                                            ptsum2[:], ptsum2[:], pt)
                                else:
                                    nc.vector.tensor_add(
                                        ptsum[:, c0:512], ptsum[:, c0:512],
                                        pt)
                            pend = (kb, c0, pt)
                            if kb == 1 and fin_prev is not None:
                                fin_prev()
                                fin_prev = None
                        if pool_side and p2_first is not None \
                                and ptsum2 is None:
                            ptsum2 = psp.tile([P, 512], f16, tag="pts2",
                                              bufs=2, name=f"p2_{qc}_{h}")
                            nc.gpsimd.tensor_copy(ptsum2[:], p2_first)
                        pkb, pc0, ppt = pend
                        nc.tensor.matmul(
                            y_ps[:, pc0:512], v_sb[:, pkb, :], ppt,
                            start=(pkb == 0), stop=True)
                        fin_prev = make_fin(h, y_ps, ptsum, ptsum2)
                    return fin_prev

                def outproj_tt(tt, defer=None, last=False):
                        ot = otp.tile([P, HID], f16, tag="ot")
                        # tail only: attention's psum banks are free, so
                        # the four groups can all stay open at once
                        OPTAGS = {0: ("pj", 3), 1: ("pj", 3),
                                  2: ("py", 2), 3: ("p1", 3)}
                        if defer is not None:
                            # all of pair 0 first: its PE work covers the
                            # deferred last-head softmax finish chain
                            opss = {}

                            def op_p0(oc):
                                tg, nb = OPTAGS[oc]
                                o_ps = ps.tile([P, 512], f32, tag=tg,
                                               bufs=nb, name=f"ops{tt}_{oc}")
                                opss[oc] = o_ps
                                op_matmuls(o_ps, tt, oc, pairs=(0,),
                                           start=True, stop=False)

                            def op_p1(oc):
                                op_matmuls(opss[oc], tt, oc, pairs=(1,),
                                           start=False, stop=True)
                                if oc % 2 == 0:
                                    nc.scalar.copy(
                                        ot[:, oc * 512:(oc + 1) * 512],
                                        opss[oc][:])
                                else:
                                    nc.vector.tensor_copy(
                                        ot[:, oc * 512:(oc + 1) * 512],
                                        opss[oc][:])

                            op_p0(0)
                            op_p0(1)
                            defer()
                            op_p0(2)
                            op_p0(3)
                            op_p1(0)
                            op_p1(1)
                            op_p1(2)
                            op_p1(3)
                        else:
                            for oc in range(4):
                                o_ps = ps.tile([P, 512], f32, tag="pj",
                                               bufs=3, name=f"ops{tt}_{oc}")
                                op_matmuls(o_ps, tt, oc)
                                if oc > 0:
                                    nc.vector.tensor_copy(
                                        ot[:, oc * 512:(oc + 1) * 512],
                                        o_ps[:])
                                else:
                                    nc.scalar.copy(
                                        ot[:, oc * 512:(oc + 1) * 512],
                                        o_ps[:])
                        if last and tt % 4 == 3:
                            for oc in range(4):
                                q = nc.sync if oc % 2 == 0 else nc.scalar
                                q.dma_start(
                                    out_d[tt * P:(tt + 1) * P,
                                          oc * 512:(oc + 1) * 512],
                                    ot[:, oc * 512:(oc + 1) * 512])
                        else:
                            nc.sync.dma_start(
                                out_d[tt * P:(tt + 1) * P, 0:1024],
                                ot[:, 0:1024])
                            nc.sync.dma_start(
                                out_d[tt * P:(tt + 1) * P, 1024:2048],
                                ot[:, 1024:2048])

                # round structure: projA/B(t) -> transposes -> prefetch
                # x(t+1) -> outproj(t-1) -> attn(t)
                OP_SCHED = [(), (0, 1, 2), (3, 4, 5), (6, 7, 8, 9, 10, 11)]
                fin = None
                for t in range(NT):
                    if 0 < t < NT - 1:
                        load_x(t + 1)
                    vt = proj_pass_a(t, defer=fin)
                    proj_pass_b1(t)
                    proj_pass_b23(t)
                    v_transposes(t, vt)
                    fin = attn_chunk(t, OP_SCHED[t])
                for tt in range((NT - 1) * 4, NT * 4):
                    outproj_tt(tt, defer=fin, last=True)
                    fin = None

            ps_cm.__exit__(None, None, None)

    nc.compile()
    return nc


def _host_consts(attention_mask):
    half = HD // 2
    inv_freq = (1.0 / (ROPE_THETA ** (np.arange(half, dtype=np.float32) / half))
                ).astype(np.float32)
    pos = np.arange(L, dtype=np.float32)
    freqs = pos[None, :] * inv_freq[:, None]          # [64, L]
    cos = np.cos(freqs).astype(np.float16)
    sin = np.sin(freqs).astype(np.float16)
    cos2 = np.ascontiguousarray(np.concatenate([cos, cos], axis=0))
    sin2 = np.ascontiguousarray(np.concatenate([sin, sin], axis=0))
    k_idx = np.arange(P)[:, None]
    q_idx = np.arange(P)[None, :]
    m01 = (k_idx <= q_idx).astype(np.float16)
    onesm = np.full((P, P), 32.0, np.float16)
    ident = np.eye(P, dtype=np.float16)
    cpack = np.ascontiguousarray(
        np.concatenate([m01, onesm, ident], axis=1))
    kbias = []
    for b in range(B):
        m = attention_mask[b].astype(np.float32)      # [L]
        bias = np.where(m > 0, 0.0, -1e4).astype(np.float32)
        kbias.append(np.ascontiguousarray(bias.reshape(TT, P).T))
    return cos2, sin2, cpack, kbias


def _hl(a):
    hi = a.astype(F8NP)
    lo = (a - hi.astype(np.float32)).astype(F8NP)
    return hi, lo


def _lay_x(a):
    """[HID, L] scaled -> [P, 2, KT, L] planes (lo, hi)."""
    hi, lo = _hl(a)

    def lay(arr):
        return arr.reshape(KT, P, L).transpose(1, 0, 2)

    return np.ascontiguousarray(np.stack([lay(lo), lay(hi)], axis=1))


def _lay_wq(a):
    """[HID, QPG*HD] scaled -> [P, 2, QPG, KT, HD] planes (hi, lo)."""
    hi, lo = _hl(a)

    def lay(arr):
        return arr.reshape(KT, P, QPG, HD).transpose(1, 2, 0, 3)

    return np.ascontiguousarray(np.stack([lay(hi), lay(lo)], axis=1))


def _lay_wkv(a):
    """[HID, 2*HD] scaled -> [P, 2, 2, KT, HD] planes (hi, lo)."""
    hi, lo = _hl(a)

    def lay(arr):
        return arr.reshape(KT, P, 2, HD).transpose(1, 2, 0, 3)

    return np.ascontiguousarray(np.stack([lay(hi), lay(lo)], axis=1))


def _lay_wo(a):
    """[QPG*HD, HID] scaled -> [P, 2, QPG, HID] planes (hi, lo)."""
    hi, lo = _hl(a)

    def lay(arr):
        return arr.reshape(QPG, P, HID).transpose(1, 0, 2)

    return np.ascontiguousarray(np.stack([lay(hi), lay(lo)], axis=1))


def kernel(x, Wq, Wc, Wk, Wv, Wo, attention_mask):
    x = np.asarray(x, dtype=np.float32)
    Wq = np.asarray(Wq, dtype=np.float32)
    Wc = np.asarray(Wc, dtype=np.float32)
    Wk = np.asarray(Wk, dtype=np.float32)
    Wv = np.asarray(Wv, dtype=np.float32)
    Wo = np.asarray(Wo, dtype=np.float32)
    attention_mask = np.asarray(attention_mask)

    if "nc" not in _CACHE:
        _CACHE["nc"] = _build()
    nc = _CACHE["nc"]

    cos2, sin2, cpack, kbias = _host_consts(attention_mask)
    # fuse the latent projection on host (exact up to fp rounding)
    Wck = (Wc.astype(np.float64) @ Wk.astype(np.float64)).astype(np.float32)
    Wcv = (Wc.astype(np.float64) @ Wv.astype(np.float64)).astype(np.float32)

    xq = [_lay_x(SX * x[b].T) for b in range(B)]

    in_maps = []
    for core in range(8):
        b, g = core // QPG, core % QPG
        wkv = np.concatenate(
            [Wck[:, g * HD:(g + 1) * HD], Wcv[:, g * HD:(g + 1) * HD]],
            axis=1)
        in_maps.append({
            "xq": xq[b],
            "wq": _lay_wq(SW * Wq[:, g * QPG * HD:(g + 1) * QPG * HD]),
            "wkv": _lay_wkv(SW * wkv),
            "wo": _lay_wo(SW * Wo[g * QPG * HD:(g + 1) * QPG * HD, :]),
            "cos2": cos2, "sin2": sin2, "cpack": cpack, "keybias": kbias[b],
        })

    res = run_bass_kernel_spmd(nc, in_maps, core_ids=list(range(8)))
    out = np.zeros((B, L, HID), dtype=np.float32)
    for core in range(8):
        out[core // QPG] += res.results[core]["out"].astype(np.float32)
    out *= 1.0 / OUT_SCALE
    return out


# revision 49
# speedup vs baseline: 1.0096x; 1.0031x over previous
"""Causal self-attention MLA (GQA, latent kv) kernel for 8 Trainium2 cores.

Sharding: the 8 cores map to (batch b, kv-group g) pairs: core = b*4 + g.
Each core computes, for its batch and its kv head (4 q-heads):
  qT = Wq_g^T x^T (rope)
  kT = (Wc Wk_g)^T x^T (rope),  vT = (Wc Wv_g)^T x^T   [latent proj fused on
      host: exact in real arithmetic]
  flash attention entirely in the transposed domain (fp16):
    ST[k,q] = kT^T qT  (per 128-k-block, causal blocks only)
    PT = exp(SCALE'*ST + keybias)
    causal zeroing of the diagonal 128x128 via fp16 0/1 mask multiply
    yT[d,q] += v[kb]^T PT
    PTsum   += PT; rowsum = (32*ones)^T PTsum via one matmul per head-chunk
  out_partial = yTn^T Wo_g  (row-parallel out proj)
Host sums the 4 partials per batch and scales by 1/512.

GEMM precision: the three dense projections (q, fused kv, out) run as
*compensated fp8e4 DoubleRow* matmuls: operands host-pre-scaled into
e4m3's sweet spot (x*4, W*64, y*8) and split into hi = fp8(a) and
lo = fp8(a - hi) planes.  Per two 128-k-tiles: 3 DR matmuls (hi*hi
packed over both k-tiles + one (hi,lo)x(lo,hi) cross DR per k-tile),
all in one fp32 PSUM group -> 0.75x the fp16 cycles at ~1.7e-3 end to
end error.  Scores and av stay fp16 (fp8 on the q/k/P path costs ~5%).

DMA: one serial DMA device in the cost model, so ORDER is everything.
Round 0 streams: SP carries x0 (hi quarters then lo), wq heads 1-3,
cos/sin, x1, wo -- FIFO gives natural back-pressure; ACT carries
k/v + q0 weights up-front (per-plane contiguous, elem>=512B, a tiny
first slice so the PE starts ~2us) and is free for PSUM evacs early;
Pool moves the tiny consts.  Projection passes emit all hi*hi mains
first (x-hi only), then cross DRs quarter-by-quarter as x-lo lands,
k's first so kT closes earliest for the rope.
"""
import numpy as np
import ml_dtypes

import concourse.bacc as bacc
import concourse.mybir as mybir
import concourse.tile as tile
from concourse.bass_utils import run_bass_kernel_spmd

B, L, HID = 2, 2048, 2048
NH, NKV, HD = 16, 4, 128
LAT = 512
QPG = NH // NKV            # q heads per kv group = 4
SCALE = float(HD) ** -0.5
ROPE_THETA = 10000.0
P = 128
NT = L // 512              # 4 token chunks of 512
KT = HID // P              # 16 contraction tiles
NG = KT // 2               # 8 DoubleRow k-tile pair groups
TT = L // P                # 16 token tiles of 128

SX, SW, SY = 4.0, 64.0, 8.0          # fp8 pre-scales (x, weights, y)
OUT_SCALE = SY * SW                  # host divides partials by this
F8NP = ml_dtypes.float8_e4m3

dt = mybir.dt
f32, f16, f8 = dt.float32, dt.float16, dt.float8e4
DR = mybir.MatmulPerfMode.DoubleRow

_CACHE = {}


def _build():
    nc = bacc.Bacc("TRN2", target_bir_lowering=False, debug=False)

    # hi/lo fp8 planes: weight-likes are [hi, lo], x-likes [lo, hi] so a
    # single (plane 0:2) AP pairs Wh*xl + Wl*xh for the cross-term DR
    xq_d = nc.dram_tensor("xq", [P, 2, KT, L], f8, kind="ExternalInput")
    wq_d = nc.dram_tensor("wq", [P, 2, QPG, KT, HD], f8,
                          kind="ExternalInput")
    wkv_d = nc.dram_tensor("wkv", [P, 2, 2, KT, HD], f8,
                           kind="ExternalInput")
    wo_d = nc.dram_tensor("wo", [P, 2, QPG, HID], f8, kind="ExternalInput")
    cos_d = nc.dram_tensor("cos2", [P, L], f16, kind="ExternalInput")
    sin_d = nc.dram_tensor("sin2", [P, L], f16, kind="ExternalInput")
    cpk_d = nc.dram_tensor("cpack", [P, 3 * P], f16, kind="ExternalInput")
    kb_d = nc.dram_tensor("keybias", [P, TT], f32, kind="ExternalInput")
    out_d = nc.dram_tensor("out", [L, HID], f16, kind="ExternalOutput")

    with tile.TileContext(nc) as tc:
        with tc.tile_pool(name="consts", bufs=1) as cp, \
             tc.tile_pool(name="qt", bufs=1) as qtp, \
             tc.tile_pool(name="yt", bufs=1) as ytp, \
             tc.tile_pool(name="kt", bufs=1) as ktp, \
             tc.tile_pool(name="vnat", bufs=1) as vnp, \
             tc.tile_pool(name="wgt", bufs=1) as wp, \
             tc.tile_pool(name="xc", bufs=2) as xp, \
             tc.tile_pool(name="ot", bufs=4) as otp:

            cos_t = cp.tile([P, L], f16)
            sin_t = cp.tile([P, L], f16)
            cpk_t = cp.tile([P, 3 * P], f16)
            kbias_t = cp.tile([P, TT], f32)
            m01_t = cpk_t[:, 0:P]          # causal 0/1 (k<=q)
            ones_t = cpk_t[:, P:2 * P]     # 32*ones (rowsum scale fold)
            idn_t = cpk_t[:, 2 * P:3 * P]

            qT = qtp.tile([P, QPG, L], f16)      # per-head 256*q, roped
            y_t = ytp.tile([P, 2, QPG, L], f8)   # normalized 8*y: [lo, hi]
            kT = ktp.tile([P, L], f16)           # kv-group 256*k, roped
            v_sb = vnp.tile([P, TT, HD], f16)    # 256*v natural [k,tile,d]
            wq_t = wp.tile([P, 2, QPG, KT, HD], f8)
            wkv_t = wp.tile([P, 2, 2, KT, HD], f8)
            wo_t = wp.tile([P, 2, QPG, HID], f8)

            # unified PSUM pool: projection + attention share the 8 banks
            ps_cm = tc.tile_pool(name="ps", bufs=8, space="PSUM")
            ps = ps_cm.__enter__()

            with tc.tile_pool(name="vt", bufs=2) as vtp, \
                 tc.tile_pool(name="rtmp", bufs=4) as rtp, \
                 tc.tile_pool(name="pt", bufs=8) as ptp, \
                 tc.tile_pool(name="pts", bufs=4) as psp, \
                 tc.tile_pool(name="rc", bufs=3) as rcp:

                x_tiles = {}

                def load_x(t, eng=None, planes=(1, 0), quarters=range(4)):
                    eng = eng or nc.sync
                    c0 = t * 512
                    if t not in x_tiles:
                        x_tiles[t] = xp.tile([P, 2, KT, 512], f8, tag="x",
                                             name=f"x{t}")
                    xt = x_tiles[t]
                    for g in quarters:
                        for pl in planes:
                            eng.dma_start(
                                xt[:, pl, 4 * g:4 * g + 4, :],
                                xq_d[:, pl, 4 * g:4 * g + 4, c0:c0 + 512])

                # ---- round-0 DMA streams (one serial DMA device: order
                # is everything; ACT must be free for evacs by ~6us) ----
                nc.gpsimd.dma_start(cpk_t[:], cpk_d[:])
                nc.gpsimd.dma_start(kbias_t[:], kb_d[:])
                # SP: x0 hi quarters (mains), x0 lo (crosses), rope consts,
                # then x1, then wo -- FIFO gives natural back-pressure
                xt0 = xp.tile([P, 2, KT, 512], f8, tag="x", name="x0")
                x_tiles[0] = xt0
                # tiny first slices so the PE's first main starts ~2us
                nc.sync.dma_start(xt0[:, 1, 0:2, :], xq_d[:, 1, 0:2, 0:512])
                nc.sync.dma_start(xt0[:, 1, 2:4, :], xq_d[:, 1, 2:4, 0:512])
                for g in range(1, 4):
                    nc.sync.dma_start(
                        xt0[:, 1, 4 * g:4 * g + 4, :],
                        xq_d[:, 1, 4 * g:4 * g + 4, 0:512])
                for g in range(4):
                    nc.sync.dma_start(
                        xt0[:, 0, 4 * g:4 * g + 4, :],
                        xq_d[:, 0, 4 * g:4 * g + 4, 0:512])
                for pl in range(2):
                    nc.sync.dma_start(wq_t[:, pl, 1], wq_d[:, pl, 1])
                nc.sync.dma_start(cos_t[:], cos_d[:])
                nc.sync.dma_start(sin_t[:], sin_d[:])
                for h in range(2, QPG):
                    for pl in range(2):
                        nc.sync.dma_start(wq_t[:, pl, h], wq_d[:, pl, h])
                load_x(1)
                for h in range(QPG):
                    for pl in range(2):
                        nc.sync.dma_start(wo_t[:, pl, h:h + 1, :],
                                          wo_d[:, pl, h:h + 1, :])
                # ACT: k/v hi weights (k kt0-1 first: the PE's first
                # operand), then rope consts (rope gates first attention),
                # then q0 hi and the lo planes
                nc.scalar.dma_start(wkv_t[:, 0, 0, 0:2], wkv_d[:, 0, 0, 0:2])
                nc.scalar.dma_start(wkv_t[:, 0, 0, 2:KT],
                                    wkv_d[:, 0, 0, 2:KT])
                nc.scalar.dma_start(wkv_t[:, 0, 1], wkv_d[:, 0, 1])
                nc.scalar.dma_start(wq_t[:, 0, 0], wq_d[:, 0, 0])
                nc.scalar.dma_start(wkv_t[:, 1], wkv_d[:, 1])
                nc.scalar.dma_start(wq_t[:, 1, 0], wq_d[:, 1, 0])

                def dr_mains(psum, w_hi, xt, start):
                    """hi*hi DR per k-tile pair; w_hi: g -> [128,2,HD] AP."""
                    for g in range(NG):
                        nc.tensor.matmul(
                            psum, w_hi(g), xt[:, 1, 2 * g:2 * g + 2, :],
                            start=(start and g == 0), stop=False,
                            perf_mode=DR)

                def dr_cross(psum, w_pair, xt, kts, stop_at=None):
                    """cross DR per k-tile; w_pair: kt -> [128,2,HD] AP."""
                    for kt in kts:
                        nc.tensor.matmul(
                            psum, w_pair(kt), xt[:, 0:2, kt, :],
                            start=False, stop=(kt == stop_at),
                            perf_mode=DR)

                def q_hi(h):
                    return lambda g: wq_t[:, 0, h, 2 * g:2 * g + 2, :]

                def q_pair(h):
                    return lambda kt: wq_t[:, 0:2, h, kt, :]

                def kv_hi(i):
                    return lambda g: wkv_t[:, 0, i, 2 * g:2 * g + 2, :]

                def kv_pair(i):
                    return lambda kt: wkv_t[:, 0:2, i, kt, :]

                def rope_chunk(dst, t, eng=None):
                    """In-place rope of dst[:, t*512:(t+1)*512] (fp16)."""
                    if eng is None:
                        eng = nc.vector
                    tg, nb = ("rt", 4) if eng is nc.vector else ("rtpool", 8)
                    c0, c1 = t * 512, (t + 1) * 512
                    t1c = rtp.tile([64, 512], f16, tag=tg, bufs=nb)
                    t1s = rtp.tile([64, 512], f16, tag=tg, bufs=nb)
                    t2c = rtp.tile([64, 512], f16, tag=tg, bufs=nb)
                    t2s = rtp.tile([64, 512], f16, tag=tg, bufs=nb)
                    eng.tensor_mul(t1c[:], dst[0:64, c0:c1], cos_t[0:64, c0:c1])
                    eng.tensor_mul(t1s[:], dst[0:64, c0:c1], sin_t[0:64, c0:c1])
                    eng.tensor_mul(t2c[:], dst[64:128, c0:c1],
                                   cos_t[64:128, c0:c1])
                    eng.tensor_mul(t2s[:], dst[64:128, c0:c1],
                                   sin_t[64:128, c0:c1])
                    eng.tensor_sub(dst[0:64, c0:c1], t1c[:], t2s[:])
                    eng.tensor_add(dst[64:128, c0:c1], t2c[:], t1s[:])

                def proj_pass_a(t, defer=None):
                    """k, v, q0: all hi*hi mains first (x-hi only), then
                    crosses quarter-by-quarter as x-lo lands; k's crosses
                    lead each quarter so kT closes earliest (rope dep)."""
                    xt = x_tiles[t]
                    c0, c1 = t * 512, (t + 1) * 512
                    kps = ps.tile([P, 512], f32, tag="pj", bufs=3,
                                  name=f"kps{t}")
                    vps = ps.tile([P, 512], f32, tag="pj", bufs=3,
                                  name=f"vps{t}")
                    q0ps = ps.tile([P, 512], f32, tag="pj", bufs=3,
                                   name=f"qA{t}")
                    for g in range(NG):
                        st = (g == 0)
                        nc.tensor.matmul(
                            kps[:], kv_hi(0)(g), xt[:, 1, 2 * g:2 * g + 2, :],
                            start=st, stop=False, perf_mode=DR)
                        nc.tensor.matmul(
                            vps[:], kv_hi(1)(g), xt[:, 1, 2 * g:2 * g + 2, :],
                            start=st, stop=False, perf_mode=DR)
                        nc.tensor.matmul(
                            q0ps[:], q_hi(0)(g), xt[:, 1, 2 * g:2 * g + 2, :],
                            start=st, stop=False, perf_mode=DR)
                        if g == 0 and defer is not None:
                            defer()
                    for q in range(4):
                        kts = range(4 * q, 4 * q + 4)
                        dr_cross(kps[:], kv_pair(0), xt, kts,
                                 stop_at=15 if q == 3 else None)
                        dr_cross(vps[:], kv_pair(1), xt, kts,
                                 stop_at=15 if q == 3 else None)
                        dr_cross(q0ps[:], q_pair(0), xt, kts,
                                 stop_at=15 if q == 3 else None)
                    nc.scalar.copy(kT[:, c0:c1], kps[:])
                    vt = vtp.tile([P, 512], f16, tag="vt")
                    nc.scalar.copy(vt[:], vps[:])
                    rope_chunk(kT, t)
                    nc.scalar.copy(qT[:, 0, c0:c1], q0ps[:])
                    rope_chunk(qT[:, 0, :], t)
                    return vt

                def proj_pass_b1(t):
                    xt = x_tiles[t]
                    c0, c1 = t * 512, (t + 1) * 512
                    qps = ps.tile([P, 512], f32, tag="pj", bufs=3,
                                  name=f"qB{t}_0")
                    dr_mains(qps[:], q_hi(1), xt, True)
                    dr_cross(qps[:], q_pair(1), xt, range(KT),
                             stop_at=KT - 1)
                    nc.scalar.copy(qT[:, 1, c0:c1], qps[:])
                    rope_chunk(qT[:, 1, :], t)

                def proj_pass_b23(t, defer=None):
                    xt = x_tiles[t]
                    c0, c1 = t * 512, (t + 1) * 512
                    qps = [ps.tile([P, 512], f32, tag="pj", bufs=3,
                                   name=f"qB{t}_{1 + i}") for i in range(2)]
                    for i, h in enumerate((2, 3)):
                        dr_mains(qps[i][:], q_hi(h), xt, True)
                        if i == 0 and defer is not None:
                            defer()
                    for i, h in enumerate((2, 3)):
                        dr_cross(qps[i][:], q_pair(h), xt, range(KT),
                                 stop_at=KT - 1)
                    for i, h in enumerate((2, 3)):
                        nc.scalar.copy(qT[:, h, c0:c1], qps[i][:])
                    rope_chunk(qT[:, 2, :], t,
                               eng=nc.vector if t == 0 else nc.gpsimd)
                    rope_chunk(qT[:, 3, :], t, eng=nc.gpsimd)

                def v_transposes(t, vt):
                    tp = ps.tile([P, 4, HD], f16, tag="p1", bufs=3,
                                 name=f"tp{t}")
                    for s in range(4):
                        nc.tensor.transpose(tp[:, s, :],
                                            vt[:, s * P:(s + 1) * P], idn_t)
                    nc.scalar.copy(v_sb[:, t * 4:t * 4 + 4, :], tp[:])

                def op_matmuls(o_ps, tt, oc, pairs=(0, 1), start=True,
                               stop=True):
                    """Out-proj DRs for head pairs into o_ps [128,512]."""
                    t0, t1 = tt * P, (tt + 1) * P
                    c0, c1 = oc * 512, (oc + 1) * 512
                    for i, p_ in enumerate(pairs):
                        h0 = 2 * p_
                        nc.tensor.matmul(
                            o_ps[:], y_t[:, 1, h0:h0 + 2, t0:t1],
                            wo_t[:, 0, h0:h0 + 2, c0:c1],
                            start=(start and i == 0), stop=False,
                            perf_mode=DR)
                        nc.tensor.matmul(
                            o_ps[:], y_t[:, 0:2, h0, t0:t1],
                            wo_t[:, 0:2, h0, c0:c1],
                            start=False, stop=False, perf_mode=DR)
                        nc.tensor.matmul(
                            o_ps[:], y_t[:, 0:2, h0 + 1, t0:t1],
                            wo_t[:, 0:2, h0 + 1, c0:c1],
                            start=False,
                            stop=(stop and i == len(pairs) - 1),
                            perf_mode=DR)

                def attn_chunk(qc, op_tts=(), heads=tuple(range(QPG)),
                               fin_in=None):
                    """Returns the deferred finisher for the last head.
                    op_tts: previous chunks' out-proj token-tiles as exp-free
                    PE filler, spread over this chunk's heads."""
                    q0 = qc * 512
                    nkb = 4 * qc + 4
                    units = [(tt, oc) for tt in op_tts for oc in range(4)]
                    if units:
                        n = len(units) // len(heads)
                        uph = {h: n for h in heads}
                    else:
                        uph = {h: 0 for h in heads}
                    ustate = {"i": 0, "ot": None, "tt": None}

                    def make_fin(h, y_ps, ptsum, ptsum2=None):
                        def fin():
                            rs_ps = ps.tile([P, 512], f32, tag="p1", bufs=3,
                                            name=f"rsps{qc}_{h}")
                            nc.tensor.matmul(rs_ps[:], ones_t, ptsum[:],
                                             start=True,
                                             stop=(ptsum2 is None))
                            if ptsum2 is not None:
                                nc.tensor.matmul(rs_ps[:], ones_t,
                                                 ptsum2[:],
                                                 start=False, stop=True)
                            rec = rcp.tile([P, 512], f32, tag="rc")
                            nc.vector.reciprocal(rec[:], rs_ps[:])
                            y16 = rcp.tile([P, 512], f16, tag="y16", bufs=2)
                            nc.vector.tensor_mul(y16[:], y_ps[:], rec[:])
                            nc.vector.tensor_copy(
                                y_t[:, 1, h, q0:q0 + 512], y16[:])
                            nc.vector.tensor_sub(
                                y_t[:, 0, h, q0:q0 + 512], y16[:],
                                y_t[:, 1, h, q0:q0 + 512])
                        return fin

                    def emit_unit():
                        i = ustate["i"]
                        if i >= len(units):
                            return
                        ustate["i"] = i + 1
                        tt, oc = units[i]
                        if oc == 0:
                            ustate["ot"] = otp.tile([P, HID], f16, tag="ot",
                                                    name=f"ot{tt}")
                        ot = ustate["ot"]
                        o_ps = ps.tile([P, 512], f32, tag="pj", bufs=3,
                                       name=f"ops{tt}_{oc}")
                        op_matmuls(o_ps, tt, oc)
                        act_heavy = (qc == NT - 1
                                     and ustate.get("h") == QPG - 1)
                        if oc == 0 or (act_heavy and oc != 2):
                            nc.scalar.copy(
                                ot[:, oc * 512:(oc + 1) * 512], o_ps[:])
                        else:
                            nc.vector.tensor_copy(
                                ot[:, oc * 512:(oc + 1) * 512], o_ps[:])
                        if oc == 1:
                            nc.sync.dma_start(
                                out_d[tt * P:(tt + 1) * P, 0:1024],
                                ot[:, 0:1024])
                        if oc == 3:
                            nc.sync.dma_start(
                                out_d[tt * P:(tt + 1) * P, 1024:2048],
                                ot[:, 1024:2048])

                    fin_prev = fin_in
                    for h in heads:
                        ustate["h"] = h
                        nph = uph[h]
                        span = (nkb - 2) if h == QPG - 1 else nkb
                        span = max(span, nph)
                        op_points = {(i + 1) * span // nph - 1: i
                                     for i in range(nph)} if nph else {}
                        y_ps = ps.tile([P, 512], f32, tag="py", bufs=2,
                                       name=f"yps{qc}_{h}")
                        ptsum = psp.tile([P, 512], f16, tag="pts")
                        # last chunk's last head: even-kb PTsum adds ride a
                        # Pool side-accumulator so the DVE clears its y-split
                        # quickly (the final out-proj waits on it)
                        pool_side = (qc == NT - 1 and h == QPG - 1)
                        ptsum2 = None
                        p2_first = None
                        pend = None
                        for kb in range(nkb):
                            c0 = max(0, kb * P - q0)
                            w = 512 - c0
                            st_ps = ps.tile([P, w], f32, tag="p1", bufs=3,
                                            name=f"stps{qc}_{h}_{kb}")
                            nc.tensor.matmul(
                                st_ps[:], kT[:, kb * P:(kb + 1) * P],
                                qT[:, h, q0 + c0:q0 + 512],
                                start=True, stop=True)
                            if kb == 0:
                                pt = ptsum[:, 0:512]
                            else:
                                ptt = ptp.tile([P, w], f16, tag="pt",
                                               name=f"pt{qc}_{h}_{kb}")
                                pt = ptt[:]
                            nc.scalar.activation(
                                pt, st_ps[:],
                                mybir.ActivationFunctionType.Exp,
                                bias=kbias_t[:, kb:kb + 1],
                                scale=SCALE / 65536.0)
                            if kb >= 4 * qc:  # diagonal: zero upper triangle
                                meng = (nc.gpsimd
                                        if qc == NT - 1 and h == 2
                                        else nc.vector)
                                meng.tensor_mul(pt[:, 0:P], pt[:, 0:P],
                                                m01_t)
                            if pend is not None:
                                pkb, pc0, ppt = pend
                                nc.tensor.matmul(
                                    y_ps[:, pc0:512], v_sb[:, pkb, :], ppt,
                                    start=(pkb == 0), stop=False)
                            if kb in op_points:
                                emit_unit()
                            if kb > 0:
                                if pool_side and kb % 2 == 0 and w == 512:
                                    if p2_first is None:
                                        p2_first = pt
                                    elif ptsum2 is None:
                                        ptsum2 = psp.tile(
                                            [P, 512], f16, tag="pts2",
                                            bufs=2, name=f"p2_{qc}_{h}")
                                        nc.gpsimd.tensor_add(
                                            ptsum2[:], p2_first, pt)
                                    else:
                                        nc.gpsimd.tensor_add(
                                            ptsum2[:], ptsum2[:], pt)
                                else:
                                    nc.vector.tensor_add(
                                        ptsum[:, c0:512], ptsum[:, c0:512],
                                        pt)
                            pend = (kb, c0, pt)
                            if kb == 1 and fin_prev is not None:
                                fin_prev()
                                fin_prev = None
                        if pool_side and p2_first is not None \
                                and ptsum2 is None:
                            ptsum2 = psp.tile([P, 512], f16, tag="pts2",
                                              bufs=2, name=f"p2_{qc}_{h}")
                            nc.gpsimd.tensor_copy(ptsum2[:], p2_first)
                        pkb, pc0, ppt = pend
                        nc.tensor.matmul(
                            y_ps[:, pc0:512], v_sb[:, pkb, :], ppt,
                            start=(pkb == 0), stop=True)
                        fin_prev = make_fin(h, y_ps, ptsum, ptsum2)
                    return fin_prev

                def outproj_tt(tt, defer=None, last=False):
                        ot = otp.tile([P, HID], f16, tag="ot")
                        # tail only: attention's psum banks are free, so
                        # the four groups can all stay open at once
                        OPTAGS = {0: ("pj", 3), 1: ("pj", 3),
                                  2: ("py", 2), 3: ("p1", 3)}
                        if defer is not None:
                            # all of pair 0 first: its PE work covers the
                            # deferred last-head softmax finish chain
                            opss = {}

                            def op_p0(oc):
                                tg, nb = OPTAGS[oc]
                                o_ps = ps.tile([P, 512], f32, tag=tg,
                                               bufs=nb, name=f"ops{tt}_{oc}")
                                opss[oc] = o_ps
                                op_matmuls(o_ps, tt, oc, pairs=(0,),
                                           start=True, stop=False)

                            def op_p1(oc):
                                op_matmuls(opss[oc], tt, oc, pairs=(1,),
                                           start=False, stop=True)
                                if oc % 2 == 0:
                                    nc.scalar.copy(
                                        ot[:, oc * 512:(oc + 1) * 512],
                                        opss[oc][:])
                                else:
                                    nc.vector.tensor_copy(
                                        ot[:, oc * 512:(oc + 1) * 512],
                                        opss[oc][:])

                            op_p0(0)
                            op_p0(1)
                            defer()
                            op_p0(2)
                            op_p0(3)
                            op_p1(0)
                            op_p1(1)
                            op_p1(2)
                            op_p1(3)
                        else:
                            for oc in range(4):
                                tg, nb = OPTAGS[oc]
                                o_ps = ps.tile([P, 512], f32, tag=tg,
                                               bufs=nb, name=f"ops{tt}_{oc}")
                                op_matmuls(o_ps, tt, oc)
                                if oc > 0:
                                    nc.vector.tensor_copy(
                                        ot[:, oc * 512:(oc + 1) * 512],
                                        o_ps[:])
                                else:
                                    nc.scalar.copy(
                                        ot[:, oc * 512:(oc + 1) * 512],
                                        o_ps[:])
                        if last and tt % 4 == 3:
                            for oc in range(4):
                                q = nc.sync if oc % 2 == 0 else nc.scalar
                                q.dma_start(
                                    out_d[tt * P:(tt + 1) * P,
                                          oc * 512:(oc + 1) * 512],
                                    ot[:, oc * 512:(oc + 1) * 512])
                        else:
                            nc.sync.dma_start(
                                out_d[tt * P:(tt + 1) * P, 0:1024],
                                ot[:, 0:1024])
                            nc.sync.dma_start(
                                out_d[tt * P:(tt + 1) * P, 1024:2048],
                                ot[:, 1024:2048])

                # round structure: projA/B(t) -> transposes -> prefetch
                # x(t+1) -> outproj(t-1) -> attn(t)
                OP_SCHED = [(), (0, 1, 2), (3, 4, 5), (6, 7, 8, 9, 10, 11)]
                fin = None
                for t in range(NT):
                    if 0 < t < NT - 1:
                        load_x(t + 1)
                    vt = proj_pass_a(t, defer=fin)
                    proj_pass_b1(t)
                    proj_pass_b23(t)
                    v_transposes(t, vt)
                    fin = attn_chunk(t, OP_SCHED[t])
                for tt in range((NT - 1) * 4, NT * 4):
                    outproj_tt(tt, defer=fin, last=True)
                    fin = None

            ps_cm.__exit__(None, None, None)

    nc.compile()
    return nc


def _host_consts(attention_mask):
    half = HD // 2
    inv_freq = (1.0 / (ROPE_THETA ** (np.arange(half, dtype=np.float32) / half))
                ).astype(np.float32)
    pos = np.arange(L, dtype=np.float32)
    freqs = pos[None, :] * inv_freq[:, None]          # [64, L]
    cos = np.cos(freqs).astype(np.float16)
    sin = np.sin(freqs).astype(np.float16)
    cos2 = np.ascontiguousarray(np.concatenate([cos, cos], axis=0))
    sin2 = np.ascontiguousarray(np.concatenate([sin, sin], axis=0))
    k_idx = np.arange(P)[:, None]
    q_idx = np.arange(P)[None, :]
    m01 = (k_idx <= q_idx).astype(np.float16)
    onesm = np.full((P, P), 32.0, np.float16)
    ident = np.eye(P, dtype=np.float16)
    cpack = np.ascontiguousarray(
        np.concatenate([m01, onesm, ident], axis=1))
    kbias = []
    for b in range(B):
        m = attention_mask[b].astype(np.float32)      # [L]
        bias = np.where(m > 0, 0.0, -1e4).astype(np.float32)
        kbias.append(np.ascontiguousarray(bias.reshape(TT, P).T))
    return cos2, sin2, cpack, kbias


def _hl(a):
    hi = a.astype(F8NP)
    lo = (a - hi.astype(np.float32)).astype(F8NP)
    return hi, lo


def _lay_x(a):
    """[HID, L] scaled -> [P, 2, KT, L] planes (lo, hi)."""
    hi, lo = _hl(a)

    def lay(arr):
        return arr.reshape(KT, P, L).transpose(1, 0, 2)

    return np.ascontiguousarray(np.stack([lay(lo), lay(hi)], axis=1))


def _lay_wq(a):
    """[HID, QPG*HD] scaled -> [P, 2, QPG, KT, HD] planes (hi, lo)."""
    hi, lo = _hl(a)

    def lay(arr):
        return arr.reshape(KT, P, QPG, HD).transpose(1, 2, 0, 3)

    return np.ascontiguousarray(np.stack([lay(hi), lay(lo)], axis=1))


def _lay_wkv(a):
    """[HID, 2*HD] scaled -> [P, 2, 2, KT, HD] planes (hi, lo)."""
    hi, lo = _hl(a)

    def lay(arr):
        return arr.reshape(KT, P, 2, HD).transpose(1, 2, 0, 3)

    return np.ascontiguousarray(np.stack([lay(hi), lay(lo)], axis=1))


def _lay_wo(a):
    """[QPG*HD, HID] scaled -> [P, 2, QPG, HID] planes (hi, lo)."""
    hi, lo = _hl(a)

    def lay(arr):
        return arr.reshape(QPG, P, HID).transpose(1, 0, 2)

    return np.ascontiguousarray(np.stack([lay(hi), lay(lo)], axis=1))


def kernel(x, Wq, Wc, Wk, Wv, Wo, attention_mask):
    x = np.asarray(x, dtype=np.float32)
    Wq = np.asarray(Wq, dtype=np.float32)
    Wc = np.asarray(Wc, dtype=np.float32)
    Wk = np.asarray(Wk, dtype=np.float32)
    Wv = np.asarray(Wv, dtype=np.float32)
    Wo = np.asarray(Wo, dtype=np.float32)
    attention_mask = np.asarray(attention_mask)

    if "nc" not in _CACHE:
        _CACHE["nc"] = _build()
    nc = _CACHE["nc"]

    cos2, sin2, cpack, kbias = _host_consts(attention_mask)
    # fuse the latent projection on host (exact up to fp rounding)
    Wck = (Wc.astype(np.float64) @ Wk.astype(np.float64)).astype(np.float32)
    Wcv = (Wc.astype(np.float64) @ Wv.astype(np.float64)).astype(np.float32)

    xq = [_lay_x(SX * x[b].T) for b in range(B)]

    in_maps = []
    for core in range(8):
        b, g = core // QPG, core % QPG
        wkv = np.concatenate(
            [Wck[:, g * HD:(g + 1) * HD], Wcv[:, g * HD:(g + 1) * HD]],
            axis=1)
        in_maps.append({
            "xq": xq[b],
            "wq": _lay_wq(SW * Wq[:, g * QPG * HD:(g + 1) * QPG * HD]),
            "wkv": _lay_wkv(SW * wkv),
            "wo": _lay_wo(SW * Wo[g * QPG * HD:(g + 1) * QPG * HD, :]),
            "cos2": cos2, "sin2": sin2, "cpack": cpack, "keybias": kbias[b],
        })

    res = run_bass_kernel_spmd(nc, in_maps, core_ids=list(range(8)))
    out = np.zeros((B, L, HID), dtype=np.float32)
    for core in range(8):
        out[core // QPG] += res.results[core]["out"].astype(np.float32)
    out *= 1.0 / OUT_SCALE
    return out
